# revision 3
# baseline (speedup 1.0000x reference)
"""
Multi-head attention Trainium2 Bass kernel (B=16, S=1024, D=768, H=12, Dh=64).

Sharding: data parallel over batch — 8 cores x 2 batches each. Weights are
replicated; no collectives.

Per-core device algorithm (all matmuls bf16 with fp32 PSUM accumulation):
  1. QK^T projection: per head-pair tiles [Q^T_h0; Q^T_h1] and [K^T_h0; K^T_h1]
     of shape [128, S] (partition = head-dim e, stacked 2 heads), computed as
     lhsT = [W_h0 | W_h1] (stationary), rhs = X^T.  bq added on the PSUM->SBUF
     copy (per-partition scalar); bk is skipped entirely (constant-per-row
     terms cancel in softmax).
  2. V projection in [t, e] layout with a zero column per head that is later
     memset to 1 (V' = [V_h | 1]) -> AV matmul also produces softmax row-sums.
  3. scores^T tiles [t, s] via row-tiled (tile_position) pairs of K=64 matmuls
     (2 heads concurrently in the 128x128 array).  Softmax without max
     subtraction (scores/8 ~ N(0,1), exp is safe in fp32): ACT exp fused with
     the PSUM->SBUF copy, scale=1/8.
  4. AV: O'^T[e|rowsum, s] = V'^T A^T accumulated over t tiles.
  5. normalize: recip(rowsum) -> partition-broadcast -> multiply -> msa^T.
  6. out-projection Y^T = Wo^T msa^T + bo' where bo' = bo + bv_flat @ Wo
     (folded on host), written to DRAM as Y^T and transposed on host.
"""

import sys

for p in ("/opt/trn_rl_repo", "/root/.axon_site/_ro/trn_rl_repo"):
    if p not in sys.path:
        sys.path.insert(0, p)

import numpy as np
import ml_dtypes

B, S, D, H, Dh = 16, 1024, 768, 12, 64
NCORE = 8
BLOC = B // NCORE          # 2 batches per core
PAIRS = H // 2             # 6 head pairs
DT = D // 128              # 6 d-tiles (contraction tiles)
TT = S // 128              # 8 t-tiles
SC = S // 512              # 2 s-chunks
VW = H * (Dh + 1)          # 780: V' width incl. ones columns

_CACHE = {}


def _build_program():
    import concourse.tile as tile
    from concourse import bacc, mybir

    bf = mybir.dt.bfloat16
    f32 = mybir.dt.float32
    EXP = mybir.ActivationFunctionType.Exp

    nc = bacc.Bacc("TRN2", target_bir_lowering=False, debug=False,
                   num_devices=NCORE)

    XT = nc.dram_tensor("XT", [BLOC, D, S], bf, kind="ExternalInput").ap()
    WQ = nc.dram_tensor("WQ", [D, D], bf, kind="ExternalInput").ap()
    WK = nc.dram_tensor("WK", [D, D], bf, kind="ExternalInput").ap()
    WV = nc.dram_tensor("WV", [D, VW], bf, kind="ExternalInput").ap()
    WO = nc.dram_tensor("WO", [D, D], bf, kind="ExternalInput").ap()
    BQ = nc.dram_tensor("BQ", [128, PAIRS], f32, kind="ExternalInput").ap()
    BO = nc.dram_tensor("BO", [128, DT], f32, kind="ExternalInput").ap()
    YT = nc.dram_tensor("YT", [BLOC, D, S], f32, kind="ExternalOutput").ap()

    with tile.TileContext(nc) as tc:
        import contextlib
        with contextlib.ExitStack() as ctx:
            consts = ctx.enter_context(tc.tile_pool(name="consts", bufs=1))
            xt_p = ctx.enter_context(tc.tile_pool(name="xt", bufs=2 * DT))
            qk_p = ctx.enter_context(tc.tile_pool(name="qk", bufs=3 * PAIRS))
            vp_p = ctx.enter_context(tc.tile_pool(name="vp", bufs=2 * TT))
            a_p = ctx.enter_context(tc.tile_pool(name="a", bufs=3))
            msa_p = ctx.enter_context(tc.tile_pool(name="msa", bufs=2 * DT))
            y_p = ctx.enter_context(tc.tile_pool(name="y", bufs=3))
            r_p = ctx.enter_context(tc.tile_pool(name="r", bufs=4))
            rb_p = ctx.enter_context(tc.tile_pool(name="rb", bufs=4))
            ps_mm = ctx.enter_context(
                tc.tile_pool(name="ps_mm", bufs=1, space="PSUM"))
            ps_sc = ctx.enter_context(
                tc.tile_pool(name="ps_sc", bufs=1, space="PSUM"))
            ps_av = ctx.enter_context(
                tc.tile_pool(name="ps_av", bufs=2, space="PSUM"))

            # ---- resident weights / biases -------------------------------
            wq_sb = []
            wk_sb = []
            wv_sb = []
            wo_sb = []
            for d in range(DT):
                t = consts.tile([128, D], bf, tag=f"wq{d}")
                nc.sync.dma_start(out=t, in_=WQ[d * 128:(d + 1) * 128, :])
                wq_sb.append(t)
                t = consts.tile([128, D], bf, tag=f"wk{d}")
                nc.sync.dma_start(out=t, in_=WK[d * 128:(d + 1) * 128, :])
                wk_sb.append(t)
                t = consts.tile([128, VW], bf, tag=f"wv{d}")
                nc.sync.dma_start(out=t, in_=WV[d * 128:(d + 1) * 128, :])
                wv_sb.append(t)
                t = consts.tile([128, D], bf, tag=f"wo{d}")
                nc.sync.dma_start(out=t, in_=WO[d * 128:(d + 1) * 128, :])
                wo_sb.append(t)
            bq_sb = consts.tile([128, PAIRS], f32, tag="bq")
            nc.sync.dma_start(out=bq_sb, in_=BQ)
            bo_sb = consts.tile([128, DT], f32, tag="bo")
            nc.sync.dma_start(out=bo_sb, in_=BO)

            for b in range(BLOC):
                # ---- load X^T tiles --------------------------------------
                xt_sb = []
                for d in range(DT):
                    t = xt_p.tile([128, S], bf, tag="xt")
                    nc.sync.dma_start(
                        out=t, in_=XT[b, d * 128:(d + 1) * 128, :])
                    xt_sb.append(t)

                # ---- QK^T projections per pair ---------------------------
                qt_sb = []
                kt_sb = []
                for p in range(PAIRS):
                    for kind in ("q", "k"):
                        w = wq_sb if kind == "q" else wk_sb
                        ps = ps_mm.tile([128, S], f32, tag="ps_mm")
                        for d in range(DT):
                            for c in range(SC):
                                nc.tensor.matmul(
                                    ps[:, c * 512:(c + 1) * 512],
                                    lhsT=w[d][:, p * 128:(p + 1) * 128],
                                    rhs=xt_sb[d][:, c * 512:(c + 1) * 512],
                                    start=(d == 0), stop=(d == DT - 1))
                        out = qk_p.tile([128, S], bf, tag="qk")
                        if kind == "q":
                            nc.vector.tensor_scalar_add(
                                out, ps, bq_sb[:, p:p + 1])
                            qt_sb.append(out)
                        else:
                            nc.vector.tensor_copy(out, ps)
                            kt_sb.append(out)

                # ---- V' in [t, e] layout (with zero cols for ones) -------
                vp_sb = []
                for T in range(TT):
                    ps = ps_mm.tile([128, S], f32, tag="ps_mm")
                    for d in range(DT):
                        nc.tensor.matmul(
                            ps[:, 0:512],
                            lhsT=xt_sb[d][:, T * 128:(T + 1) * 128],
                            rhs=wv_sb[d][:, 0:512],
                            start=(d == 0), stop=(d == DT - 1))
                        nc.tensor.matmul(
                            ps[:, 512:VW],
                            lhsT=xt_sb[d][:, T * 128:(T + 1) * 128],
                            rhs=wv_sb[d][:, 512:VW],
                            start=(d == 0), stop=(d == DT - 1))
                    vp = vp_p.tile([128, VW], bf, tag="vp")
                    nc.vector.tensor_copy(vp, ps[:, 0:VW])
                    v3 = vp.rearrange("p (h e) -> p h e", e=Dh + 1)
                    nc.vector.memset(v3[:, :, Dh:Dh + 1], 1.0)
                    vp_sb.append(vp)

                # ---- attention per pair ----------------------------------
                msa_sb = []
                for p in range(PAIRS):
                    msa = msa_p.tile([128, S], bf, tag="msa")
                    msa_sb.append(msa)
                    for c in range(SC):
                        po = [ps_av.tile([65, 512], f32, tag="ps_av",
                                         name=f"po{h}")
                              for h in range(2)]
                        for Tp in range(TT // 2):
                            pss = ps_sc.tile([128, 2048], f32, tag="ps_sc")
                            for j in range(2):
                                T = 2 * Tp + j
                                for h in range(2):
                                    nc.tensor.matmul(
                                        pss[:, (2 * j + h) * 512:
                                            (2 * j + h + 1) * 512],
                                        lhsT=kt_sb[p][h * 64:(h + 1) * 64,
                                                      T * 128:(T + 1) * 128],
                                        rhs=qt_sb[p][h * 64:(h + 1) * 64,
                                                     c * 512:(c + 1) * 512],
                                        start=True, stop=True,
                                        tile_position=(h * 64, 0))
                            at = a_p.tile([128, 2048], bf, tag="a")
                            nc.scalar.activation(at, pss, EXP, scale=0.125)
                            for j in range(2):
                                T = 2 * Tp + j
                                for h in range(2):
                                    nc.tensor.matmul(
                                        po[h],
                                        lhsT=vp_sb[T][
                                            :, (2 * p + h) * (Dh + 1):
                                            (2 * p + h + 1) * (Dh + 1)],
                                        rhs=at[:, (2 * j + h) * 512:
                                               (2 * j + h + 1) * 512],
                                        start=(T == 0), stop=(T == TT - 1))
                        for h in range(2):
                            r = r_p.tile([1, 512], f32, tag="r")
                            nc.vector.reciprocal(r, po[h][64:65, :])
                            rb = rb_p.tile([64, 512], f32, tag="rb")
                            nc.gpsimd.partition_broadcast(rb, r)
                            nc.vector.tensor_mul(
                                msa[h * 64:(h + 1) * 64,
                                    c * 512:(c + 1) * 512],
                                po[h][0:64, :], rb)

                # ---- output projection -----------------------------------
                for o in range(DT):
                    ps = ps_mm.tile([128, S], f32, tag="ps_mm")
                    for d in range(DT):
                        for c in range(SC):
                            nc.tensor.matmul(
                                ps[:, c * 512:(c + 1) * 512],
                                lhsT=wo_sb[d][:, o * 128:(o + 1) * 128],
                                rhs=msa_sb[d][:, c * 512:(c + 1) * 512],
                                start=(d == 0), stop=(d == DT - 1))
                    y = y_p.tile([128, S], f32, tag="y")
                    nc.vector.tensor_scalar_add(y, ps, bo_sb[:, o:o + 1])
                    nc.sync.dma_start(
                        out=YT[b, o * 128:(o + 1) * 128, :], in_=y)

    nc.compile()
    return nc


def _prep_inputs(X, Wq, bq, Wk, bk, Wv, bv, Wo, bo):
    bf16 = ml_dtypes.bfloat16
    X = np.asarray(X, dtype=np.float32)
    # per-core X^T: [core][BLOC, D, S]
    xt = np.ascontiguousarray(
        X.reshape(NCORE, BLOC, S, D).transpose(0, 1, 3, 2)).astype(bf16)
    wq = np.ascontiguousarray(
        np.asarray(Wq, np.float32).transpose(1, 0, 2).reshape(D, D)).astype(bf16)
    wk = np.ascontiguousarray(
        np.asarray(Wk, np.float32).transpose(1, 0, 2).reshape(D, D)).astype(bf16)
    wv = np.zeros((D, VW), np.float32)
    Wv = np.asarray(Wv, np.float32)
    for h in range(H):
        wv[:, h * (Dh + 1):h * (Dh + 1) + Dh] = Wv[h]
    wv = wv.astype(bf16)
    wo = np.asarray(Wo, np.float32).astype(bf16)
    bq2 = np.ascontiguousarray(
        np.asarray(bq, np.float32).reshape(PAIRS, 128).T)
    bo_eff = np.asarray(bo, np.float32) + \
        np.asarray(bv, np.float32).reshape(-1) @ np.asarray(Wo, np.float32)
    bo2 = np.ascontiguousarray(bo_eff.reshape(DT, 128).T.astype(np.float32))
    in_maps = [
        {"XT": xt[c], "WQ": wq, "WK": wk, "WV": wv, "WO": wo,
         "BQ": bq2, "BO": bo2}
        for c in range(NCORE)
    ]
    return in_maps


def _get_runner():
    """Build (once) a jitted SPMD runner over the 8 cores, modeled on
    bass2jax.run_bass_via_pjrt but cached so repeat calls don't re-trace."""
    if "runner" in _CACHE:
        return _CACHE["runner"]

    import jax
    import numpy as _np
    from jax.sharding import Mesh, PartitionSpec, NamedSharding
    from jax.experimental.shard_map import shard_map
    from concourse import mybir
    from concourse.bass2jax import (
        _bass_exec_p, install_neuronx_cc_hook, partition_id_tensor)

    nc = _build_program()
    install_neuronx_cc_hook()

    import concourse.mybir as _mybir
    in_names, out_names, out_avals, zero_shapes = [], [], [], []
    partition_name = (nc.partition_id_tensor.name
                      if nc.partition_id_tensor else None)
    for alloc in nc.m.functions[0].allocations:
        if not isinstance(alloc, _mybir.MemoryLocationSet):
            continue
        name = alloc.memorylocations[0].name
        if alloc.kind == "ExternalInput":
            if name != partition_name:
                in_names.append(name)
        elif alloc.kind == "ExternalOutput":
            shape = tuple(alloc.tensor_shape)
            dtype = _mybir.dt.np(alloc.dtype)
            out_names.append(name)
            out_avals.append(jax.core.ShapedArray(shape, dtype))
            zero_shapes.append((shape, dtype))
    n_params = len(in_names)
    n_outs = len(out_names)
    all_in_names = in_names + out_names
    if partition_name is not None:
        all_in_names = all_in_names + [partition_name]

    def _body(*args):
        operands = list(args)
        if partition_name is not None:
            operands.append(partition_id_tensor())
        outs = _bass_exec_p.bind(
            *operands,
            out_avals=tuple(out_avals),
            in_names=tuple(all_in_names),
            out_names=tuple(out_names),
            lowering_input_output_aliases=(),
            sim_require_finite=True,
            sim_require_nnan=True,
            nc=nc,
        )
        return tuple(outs)

    devices = jax.devices()[:NCORE]
    mesh = Mesh(_np.asarray(devices), ("core",))
    in_specs = (PartitionSpec("core"),) * (n_params + n_outs)
    out_specs = (PartitionSpec("core"),) * n_outs
    donate = tuple(range(n_params, n_params + n_outs))
    sharded = jax.jit(
        shard_map(_body, mesh=mesh, in_specs=in_specs, out_specs=out_specs,
                  check_rep=False),
        donate_argnums=donate, keep_unused=True)
    shard = NamedSharding(mesh, PartitionSpec("core"))

    def make_zeros():
        import jax.numpy as jnp
        return [jnp.zeros((NCORE * s[0], *s[1:]), d, device=shard)
                for s, d in zero_shapes]

    def put_inputs(in_maps):
        # concatenate along axis 0 (per-core stacking)
        concat = []
        for nm in in_names:
            arrs = [_np.asarray(in_maps[c][nm]) for c in range(NCORE)]
            concat.append(_np.concatenate(arrs, axis=0))
        return [jax.device_put(a, shard) for a in concat]

    def run(dev_inputs):
        outs = sharded(*dev_inputs, *make_zeros())
        jax.block_until_ready(outs)
        return outs

    def unpack(outs):
        res = []
        for c in range(NCORE):
            d = {}
            for i, nm in enumerate(out_names):
                full = _np.asarray(outs[i])
                d[nm] = full.reshape(NCORE, *out_avals[i].shape)[c]
            res.append(d)
        return res

    _CACHE["runner"] = (put_inputs, run, unpack)
    return _CACHE["runner"]


def kernel(X, Wq, bq, Wk, bk, Wv, bv, Wo, bo):
    put_inputs, run, unpack = _get_runner()
    in_maps = _prep_inputs(X, Wq, bq, Wk, bk, Wv, bv, Wo, bo)
    dev_inputs = put_inputs(in_maps)
    outs = run(dev_inputs)
    res = unpack(outs)
    y = np.concatenate(
        [r["YT"].transpose(0, 2, 1) for r in res], axis=0)
    return np.ascontiguousarray(y.astype(np.float32))


# revision 9
# speedup vs baseline: 1.1260x; 1.1260x over previous
"""
Multi-head attention Trainium2 Bass kernel (B=16, S=1024, D=768, H=12, Dh=64).

Sharding: data parallel over batch — 8 cores x 2 batches each. Weights are
replicated; no collectives.

Per-core device algorithm (all matmuls bf16 with fp32 PSUM accumulation):
  1. QK^T projection: per head-pair tiles [Q^T_h0; Q^T_h1] and [K^T_h0; K^T_h1]
     of shape [128, S] (partition = head-dim e, stacked 2 heads), computed as
     lhsT = [W_h0 | W_h1] (stationary), rhs = X^T.  bq added on the PSUM->SBUF
     copy (per-partition scalar); bk is skipped entirely (constant-per-row
     terms cancel in softmax).
  2. V projection in [t, e] layout with a zero column per head that is later
     memset to 1 (V' = [V_h | 1]) -> AV matmul also produces softmax row-sums.
  3. scores^T tiles [t, s] via row-tiled (tile_position) pairs of K=64 matmuls
     (2 heads concurrently in the 128x128 array).  Softmax without max
     subtraction (scores/8 ~ N(0,1), exp is safe in fp32): ACT exp fused with
     the PSUM->SBUF copy, scale=1/8.
  4. AV: O'^T[e|rowsum, s] = V'^T A^T accumulated over t tiles.
  5. normalize: recip(rowsum) -> partition-broadcast -> multiply -> msa^T.
  6. out-projection Y^T = Wo^T msa^T + bo' where bo' = bo + bv_flat @ Wo
     (folded on host), written to DRAM as Y^T and transposed on host.
"""

import sys

for p in ("/opt/trn_rl_repo", "/root/.axon_site/_ro/trn_rl_repo"):
    if p not in sys.path:
        sys.path.insert(0, p)

import numpy as np
import ml_dtypes

B, S, D, H, Dh = 16, 1024, 768, 12, 64
NCORE = 8
BLOC = B // NCORE          # 2 batches per core
PAIRS = H // 2             # 6 head pairs
DT = D // 128              # 6 d-tiles (contraction tiles)
TT = S // 128              # 8 t-tiles
SC = S // 512              # 2 s-chunks
VW = H * (Dh + 1)          # 780: V' width incl. ones columns

_CACHE = {}


def _build_program():
    import concourse.tile as tile
    from concourse import bacc, mybir

    bf = mybir.dt.bfloat16
    f32 = mybir.dt.float32
    EXP = mybir.ActivationFunctionType.Exp

    nc = bacc.Bacc("TRN2", target_bir_lowering=False, debug=False,
                   num_devices=NCORE)

    XT = nc.dram_tensor("XT", [BLOC, D, S], bf, kind="ExternalInput").ap()
    WQ = nc.dram_tensor("WQ", [D, D], bf, kind="ExternalInput").ap()
    WK = nc.dram_tensor("WK", [D, D], bf, kind="ExternalInput").ap()
    WV = nc.dram_tensor("WV", [D, VW], bf, kind="ExternalInput").ap()
    WO = nc.dram_tensor("WO", [D, D], bf, kind="ExternalInput").ap()
    BQ = nc.dram_tensor("BQ", [128, PAIRS], f32, kind="ExternalInput").ap()
    BO = nc.dram_tensor("BO", [128, DT], f32, kind="ExternalInput").ap()
    YT = nc.dram_tensor("YT", [BLOC, D, S], f32, kind="ExternalOutput").ap()

    with tile.TileContext(nc) as tc:
        import contextlib
        with contextlib.ExitStack() as ctx:
            consts = ctx.enter_context(tc.tile_pool(name="consts", bufs=1))
            xt_p = ctx.enter_context(tc.tile_pool(name="xt", bufs=2 * DT))
            qk_p = ctx.enter_context(tc.tile_pool(name="qk", bufs=3 * PAIRS))
            vp_p = ctx.enter_context(tc.tile_pool(name="vp", bufs=2 * TT))
            a_p = ctx.enter_context(tc.tile_pool(name="a", bufs=3))
            msa_p = ctx.enter_context(tc.tile_pool(name="msa", bufs=2 * DT))
            y_p = ctx.enter_context(tc.tile_pool(name="y", bufs=3))
            r_p = ctx.enter_context(tc.tile_pool(name="r", bufs=4))
            rb_p = ctx.enter_context(tc.tile_pool(name="rb", bufs=4))
            ps_mm = ctx.enter_context(
                tc.tile_pool(name="ps_mm", bufs=1, space="PSUM"))
            ps_sc = ctx.enter_context(
                tc.tile_pool(name="ps_sc", bufs=1, space="PSUM"))
            ps_av = ctx.enter_context(
                tc.tile_pool(name="ps_av", bufs=2, space="PSUM"))

            # ---- resident weights / biases -------------------------------
            wq_sb = []
            wk_sb = []
            wv_sb = []
            wo_sb = []
            for d in range(DT):
                t = consts.tile([128, D], bf, tag=f"wq{d}")
                nc.sync.dma_start(out=t, in_=WQ[d * 128:(d + 1) * 128, :])
                wq_sb.append(t)
                t = consts.tile([128, D], bf, tag=f"wk{d}")
                nc.sync.dma_start(out=t, in_=WK[d * 128:(d + 1) * 128, :])
                wk_sb.append(t)
                t = consts.tile([128, VW], bf, tag=f"wv{d}")
                nc.sync.dma_start(out=t, in_=WV[d * 128:(d + 1) * 128, :])
                wv_sb.append(t)
                t = consts.tile([128, D], bf, tag=f"wo{d}")
                nc.sync.dma_start(out=t, in_=WO[d * 128:(d + 1) * 128, :])
                wo_sb.append(t)
            bq_sb = consts.tile([128, PAIRS], f32, tag="bq")
            nc.sync.dma_start(out=bq_sb, in_=BQ)
            bo_sb = consts.tile([128, DT], f32, tag="bo")
            nc.sync.dma_start(out=bo_sb, in_=BO)

            for b in range(BLOC):
                # ---- load X^T tiles --------------------------------------
                xt_sb = []
                for d in range(DT):
                    t = xt_p.tile([128, S], bf, tag="xt")
                    nc.sync.dma_start(
                        out=t, in_=XT[b, d * 128:(d + 1) * 128, :])
                    xt_sb.append(t)

                # ---- QK^T projections per pair ---------------------------
                qt_sb = []
                kt_sb = []
                for p in range(PAIRS):
                    for kind in ("q", "k"):
                        w = wq_sb if kind == "q" else wk_sb
                        ps = ps_mm.tile([128, S], f32, tag="ps_mm")
                        for d in range(DT):
                            for c in range(SC):
                                nc.tensor.matmul(
                                    ps[:, c * 512:(c + 1) * 512],
                                    lhsT=w[d][:, p * 128:(p + 1) * 128],
                                    rhs=xt_sb[d][:, c * 512:(c + 1) * 512],
                                    start=(d == 0), stop=(d == DT - 1))
                        out = qk_p.tile([128, S], bf, tag="qk")
                        if kind == "q":
                            nc.vector.tensor_scalar_add(
                                out, ps, bq_sb[:, p:p + 1])
                            qt_sb.append(out)
                        else:
                            nc.vector.tensor_copy(out, ps)
                            kt_sb.append(out)

                # ---- V' in [t, e] layout (with zero cols for ones) -------
                vp_sb = []
                for T in range(TT):
                    ps = ps_mm.tile([128, S], f32, tag="ps_mm")
                    for d in range(DT):
                        nc.tensor.matmul(
                            ps[:, 0:512],
                            lhsT=xt_sb[d][:, T * 128:(T + 1) * 128],
                            rhs=wv_sb[d][:, 0:512],
                            start=(d == 0), stop=(d == DT - 1))
                        nc.tensor.matmul(
                            ps[:, 512:VW],
                            lhsT=xt_sb[d][:, T * 128:(T + 1) * 128],
                            rhs=wv_sb[d][:, 512:VW],
                            start=(d == 0), stop=(d == DT - 1))
                    vp = vp_p.tile([128, VW], bf, tag="vp")
                    nc.vector.tensor_copy(vp, ps[:, 0:VW])
                    v3 = vp.rearrange("p (h e) -> p h e", e=Dh + 1)
                    nc.vector.memset(v3[:, :, Dh:Dh + 1], 1.0)
                    vp_sb.append(vp)

                # ---- attention per pair ----------------------------------
                msa_sb = []
                for p in range(PAIRS):
                    msa = msa_p.tile([128, S], bf, tag="msa")
                    msa_sb.append(msa)
                    for c in range(SC):
                        po = [ps_av.tile([65, 512], f32, tag="ps_av",
                                         name=f"po{h}")
                              for h in range(2)]
                        for Tp in range(TT // 2):
                            pss = ps_sc.tile([128, 2048], f32, tag="ps_sc")
                            for j in range(2):
                                T = 2 * Tp + j
                                for h in range(2):
                                    nc.tensor.matmul(
                                        pss[:, (2 * j + h) * 512:
                                            (2 * j + h + 1) * 512],
                                        lhsT=kt_sb[p][h * 64:(h + 1) * 64,
                                                      T * 128:(T + 1) * 128],
                                        rhs=qt_sb[p][h * 64:(h + 1) * 64,
                                                     c * 512:(c + 1) * 512],
                                        start=True, stop=True,
                                        tile_position=(h * 64, 0))
                            at = a_p.tile([128, 2048], bf, tag="a")
                            nc.scalar.activation(at, pss, EXP, scale=0.125)
                            for j in range(2):
                                T = 2 * Tp + j
                                for h in range(2):
                                    nc.tensor.matmul(
                                        po[h],
                                        lhsT=vp_sb[T][
                                            :, (2 * p + h) * (Dh + 1):
                                            (2 * p + h + 1) * (Dh + 1)],
                                        rhs=at[:, (2 * j + h) * 512:
                                               (2 * j + h + 1) * 512],
                                        start=(T == 0), stop=(T == TT - 1))
                        for h in range(2):
                            r = r_p.tile([1, 512], f32, tag="r")
                            nc.vector.reciprocal(r, po[h][64:65, :])
                            rb = rb_p.tile([64, 512], f32, tag="rb")
                            nc.gpsimd.partition_broadcast(rb, r)
                            nc.vector.tensor_mul(
                                msa[h * 64:(h + 1) * 64,
                                    c * 512:(c + 1) * 512],
                                po[h][0:64, :], rb)

                # ---- output projection -----------------------------------
                for o in range(DT):
                    ps = ps_mm.tile([128, S], f32, tag="ps_mm")
                    for d in range(DT):
                        for c in range(SC):
                            nc.tensor.matmul(
                                ps[:, c * 512:(c + 1) * 512],
                                lhsT=wo_sb[d][:, o * 128:(o + 1) * 128],
                                rhs=msa_sb[d][:, c * 512:(c + 1) * 512],
                                start=(d == 0), stop=(d == DT - 1))
                    y = y_p.tile([128, S], f32, tag="y")
                    nc.vector.tensor_scalar_add(y, ps, bo_sb[:, o:o + 1])
                    nc.sync.dma_start(
                        out=YT[b, o * 128:(o + 1) * 128, :], in_=y)

    nc.compile()
    return nc


def _prep_inputs(X, Wq, bq, Wk, bk, Wv, bv, Wo, bo):
    bf16 = ml_dtypes.bfloat16
    X = np.asarray(X, dtype=np.float32)
    # per-core X^T: [core][BLOC, D, S]
    xt = np.ascontiguousarray(
        X.reshape(NCORE, BLOC, S, D).transpose(0, 1, 3, 2)).astype(bf16)
    wq = np.ascontiguousarray(
        np.asarray(Wq, np.float32).transpose(1, 0, 2).reshape(D, D)).astype(bf16)
    wk = np.ascontiguousarray(
        np.asarray(Wk, np.float32).transpose(1, 0, 2).reshape(D, D)).astype(bf16)
    wv = np.zeros((D, VW), np.float32)
    Wv = np.asarray(Wv, np.float32)
    for h in range(H):
        wv[:, h * (Dh + 1):h * (Dh + 1) + Dh] = Wv[h]
    wv = wv.astype(bf16)
    wo = np.asarray(Wo, np.float32).astype(bf16)
    bq2 = np.ascontiguousarray(
        np.asarray(bq, np.float32).reshape(PAIRS, 128).T)
    bo_eff = np.asarray(bo, np.float32) + \
        np.asarray(bv, np.float32).reshape(-1) @ np.asarray(Wo, np.float32)
    bo2 = np.ascontiguousarray(bo_eff.reshape(DT, 128).T.astype(np.float32))
    in_maps = [
        {"XT": xt[c], "WQ": wq, "WK": wk, "WV": wv, "WO": wo,
         "BQ": bq2, "BO": bo2}
        for c in range(NCORE)
    ]
    return in_maps


def _get_runner():
    """Build (once) a jitted SPMD runner over the 8 cores, modeled on
    bass2jax.run_bass_via_pjrt but cached so repeat calls don't re-trace."""
    if "runner" in _CACHE:
        return _CACHE["runner"]

    import jax
    import numpy as _np
    from jax.sharding import Mesh, PartitionSpec, NamedSharding
    from jax.experimental.shard_map import shard_map
    from concourse import mybir
    from concourse.bass2jax import (
        _bass_exec_p, install_neuronx_cc_hook, partition_id_tensor)

    nc = _build_program()
    install_neuronx_cc_hook()

    import concourse.mybir as _mybir
    in_names, out_names, out_avals, zero_shapes = [], [], [], []
    partition_name = (nc.partition_id_tensor.name
                      if nc.partition_id_tensor else None)
    for alloc in nc.m.functions[0].allocations:
        if not isinstance(alloc, _mybir.MemoryLocationSet):
            continue
        name = alloc.memorylocations[0].name
        if alloc.kind == "ExternalInput":
            if name != partition_name:
                in_names.append(name)
        elif alloc.kind == "ExternalOutput":
            shape = tuple(alloc.tensor_shape)
            dtype = _mybir.dt.np(alloc.dtype)
            out_names.append(name)
            out_avals.append(jax.core.ShapedArray(shape, dtype))
            zero_shapes.append((shape, dtype))
    n_params = len(in_names)
    n_outs = len(out_names)
    all_in_names = in_names + out_names
    if partition_name is not None:
        all_in_names = all_in_names + [partition_name]

    def _body(*args):
        operands = list(args)
        if partition_name is not None:
            operands.append(partition_id_tensor())
        outs = _bass_exec_p.bind(
            *operands,
            out_avals=tuple(out_avals),
            in_names=tuple(all_in_names),
            out_names=tuple(out_names),
            lowering_input_output_aliases=(),
            sim_require_finite=True,
            sim_require_nnan=True,
            nc=nc,
        )
        return tuple(outs)

    devices = jax.devices()[:NCORE]
    mesh = Mesh(_np.asarray(devices), ("core",))
    in_specs = (PartitionSpec("core"),) * (n_params + n_outs)
    out_specs = (PartitionSpec("core"),) * n_outs
    # NOTE: no donation — the kernel writes every output element, so the
    # custom call's self-allocated (uninit) output buffers are fine, and the
    # zero "output operand" arrays can be created once and reused across
    # calls instead of being shipped host->device (50 MB) per call.
    sharded = jax.jit(
        shard_map(_body, mesh=mesh, in_specs=in_specs, out_specs=out_specs,
                  check_rep=False),
        keep_unused=True)
    shard = NamedSharding(mesh, PartitionSpec("core"))
    import jax.numpy as jnp
    zeros_dev = [
        jax.device_put(_np.zeros((NCORE * s[0], *s[1:]), d), shard)
        for s, d in zero_shapes
    ]

    def put_inputs(in_maps):
        # concatenate along axis 0 (per-core stacking)
        concat = []
        for nm in in_names:
            arrs = [_np.asarray(in_maps[c][nm]) for c in range(NCORE)]
            concat.append(_np.concatenate(arrs, axis=0))
        return [jax.device_put(a, shard) for a in concat]

    def run(dev_inputs):
        outs = sharded(*dev_inputs, *zeros_dev)
        jax.block_until_ready(outs)
        return outs

    def unpack(outs):
        res = []
        for c in range(NCORE):
            d = {}
            for i, nm in enumerate(out_names):
                full = _np.asarray(outs[i])
                d[nm] = full.reshape(NCORE, *out_avals[i].shape)[c]
            res.append(d)
        return res

    _CACHE["runner"] = (put_inputs, run, unpack)
    return _CACHE["runner"]


def kernel(X, Wq, bq, Wk, bk, Wv, bv, Wo, bo):
    put_inputs, run, unpack = _get_runner()
    in_maps = _prep_inputs(X, Wq, bq, Wk, bk, Wv, bv, Wo, bo)
    dev_inputs = put_inputs(in_maps)
    outs = run(dev_inputs)
    res = unpack(outs)
    y = np.concatenate(
        [r["YT"].transpose(0, 2, 1) for r in res], axis=0)
    return np.ascontiguousarray(y.astype(np.float32))


# revision 14
# speedup vs baseline: 222.4894x; 197.5922x over previous
"""
Multi-head attention Trainium2 Bass kernel (B=16, S=1024, D=768, H=12, Dh=64).

Sharding: data parallel over batch — 8 cores x 2 batches each. Weights are
replicated; no collectives.

Per-core device algorithm (all matmuls bf16 with fp32 PSUM accumulation):
  1. QK^T projection: per head-pair tiles [Q^T_h0; Q^T_h1] and [K^T_h0; K^T_h1]
     of shape [128, S] (partition = head-dim e, stacked 2 heads), computed as
     lhsT = [W_h0 | W_h1] (stationary), rhs = X^T.  bq added on the PSUM->SBUF
     copy (per-partition scalar); bk is skipped entirely (constant-per-row
     terms cancel in softmax).
  2. V projection in [t, e] layout with a zero column per head that is later
     memset to 1 (V' = [V_h | 1]) -> AV matmul also produces softmax row-sums.
  3. scores^T tiles [t, s] via row-tiled (tile_position) pairs of K=64 matmuls
     (2 heads concurrently in the 128x128 array).  Softmax without max
     subtraction (scores/8 ~ N(0,1), exp is safe in fp32): ACT exp fused with
     the PSUM->SBUF copy, scale=1/8.
  4. AV: O'^T[e|rowsum, s] = V'^T A^T accumulated over t tiles.
  5. normalize: recip(rowsum) -> partition-broadcast -> multiply -> msa^T.
  6. out-projection Y^T = Wo^T msa^T + bo' where bo' = bo + bv_flat @ Wo
     (folded on host), written to DRAM as Y^T and transposed on host.
"""

import sys

for p in ("/opt/trn_rl_repo", "/root/.axon_site/_ro/trn_rl_repo"):
    if p not in sys.path:
        sys.path.insert(0, p)

import numpy as np
import ml_dtypes

B, S, D, H, Dh = 16, 1024, 768, 12, 64
NCORE = 8
BLOC = B // NCORE          # 2 batches per core
PAIRS = H // 2             # 6 head pairs
DT = D // 128              # 6 d-tiles (contraction tiles)
TT = S // 128              # 8 t-tiles
SC = S // 512              # 2 s-chunks
VW = H * (Dh + 1)          # 780: V' width incl. ones columns

_CACHE = {}


def _build_program(repeats=1):
    import concourse.tile as tile
    from concourse import bacc, mybir

    bf = mybir.dt.bfloat16
    f32 = mybir.dt.float32
    EXP = mybir.ActivationFunctionType.Exp

    nc = bacc.Bacc("TRN2", target_bir_lowering=False, debug=False,
                   num_devices=NCORE)

    XT = nc.dram_tensor("XT", [BLOC, D, S], bf, kind="ExternalInput").ap()
    WQ = nc.dram_tensor("WQ", [D, D], bf, kind="ExternalInput").ap()
    WK = nc.dram_tensor("WK", [D, D], bf, kind="ExternalInput").ap()
    WV = nc.dram_tensor("WV", [D, VW], bf, kind="ExternalInput").ap()
    WO = nc.dram_tensor("WO", [D, D], bf, kind="ExternalInput").ap()
    BQ = nc.dram_tensor("BQ", [128, PAIRS], f32, kind="ExternalInput").ap()
    BO = nc.dram_tensor("BO", [128, DT], f32, kind="ExternalInput").ap()
    YT = nc.dram_tensor("YT", [BLOC, D, S], f32, kind="ExternalOutput").ap()

    with tile.TileContext(nc) as tc:
        import contextlib
        with contextlib.ExitStack() as ctx:
            consts = ctx.enter_context(tc.tile_pool(name="consts", bufs=1))
            xt_p = ctx.enter_context(tc.tile_pool(name="xt", bufs=2 * DT))
            qk_p = ctx.enter_context(tc.tile_pool(name="qk", bufs=3 * PAIRS))
            vp_p = ctx.enter_context(tc.tile_pool(name="vp", bufs=2 * TT))
            a_p = ctx.enter_context(tc.tile_pool(name="a", bufs=3))
            msa_p = ctx.enter_context(tc.tile_pool(name="msa", bufs=2 * DT))
            y_p = ctx.enter_context(tc.tile_pool(name="y", bufs=3))
            r_p = ctx.enter_context(tc.tile_pool(name="r", bufs=4))
            rb_p = ctx.enter_context(tc.tile_pool(name="rb", bufs=4))
            ps_mm = ctx.enter_context(
                tc.tile_pool(name="ps_mm", bufs=1, space="PSUM"))
            ps_sc = ctx.enter_context(
                tc.tile_pool(name="ps_sc", bufs=1, space="PSUM"))
            ps_av = ctx.enter_context(
                tc.tile_pool(name="ps_av", bufs=2, space="PSUM"))

            # ---- resident weights / biases -------------------------------
            wq_sb = []
            wk_sb = []
            wv_sb = []
            wo_sb = []
            for d in range(DT):
                t = consts.tile([128, D], bf, tag=f"wq{d}")
                nc.sync.dma_start(out=t, in_=WQ[d * 128:(d + 1) * 128, :])
                wq_sb.append(t)
                t = consts.tile([128, D], bf, tag=f"wk{d}")
                nc.sync.dma_start(out=t, in_=WK[d * 128:(d + 1) * 128, :])
                wk_sb.append(t)
                t = consts.tile([128, VW], bf, tag=f"wv{d}")
                nc.sync.dma_start(out=t, in_=WV[d * 128:(d + 1) * 128, :])
                wv_sb.append(t)
                t = consts.tile([128, D], bf, tag=f"wo{d}")
                nc.sync.dma_start(out=t, in_=WO[d * 128:(d + 1) * 128, :])
                wo_sb.append(t)
            bq_sb = consts.tile([128, PAIRS], f32, tag="bq")
            nc.sync.dma_start(out=bq_sb, in_=BQ)
            bo_sb = consts.tile([128, DT], f32, tag="bo")
            nc.sync.dma_start(out=bo_sb, in_=BO)

            for b in [b for _ in range(repeats) for b in range(BLOC)]:
                # ---- load X^T tiles --------------------------------------
                xt_sb = []
                for d in range(DT):
                    t = xt_p.tile([128, S], bf, tag="xt")
                    nc.sync.dma_start(
                        out=t, in_=XT[b, d * 128:(d + 1) * 128, :])
                    xt_sb.append(t)

                # ---- QK^T projections per pair ---------------------------
                qt_sb = []
                kt_sb = []
                for p in range(PAIRS):
                    for kind in ("q", "k"):
                        w = wq_sb if kind == "q" else wk_sb
                        ps = ps_mm.tile([128, S], f32, tag="ps_mm")
                        for d in range(DT):
                            for c in range(SC):
                                nc.tensor.matmul(
                                    ps[:, c * 512:(c + 1) * 512],
                                    lhsT=w[d][:, p * 128:(p + 1) * 128],
                                    rhs=xt_sb[d][:, c * 512:(c + 1) * 512],
                                    start=(d == 0), stop=(d == DT - 1))
                        out = qk_p.tile([128, S], bf, tag="qk")
                        if kind == "q":
                            nc.vector.tensor_scalar_add(
                                out, ps, bq_sb[:, p:p + 1])
                            qt_sb.append(out)
                        else:
                            nc.vector.tensor_copy(out, ps)
                            kt_sb.append(out)

                # ---- V' in [t, e] layout (with zero cols for ones) -------
                vp_sb = []
                for T in range(TT):
                    ps = ps_mm.tile([128, S], f32, tag="ps_mm")
                    for d in range(DT):
                        nc.tensor.matmul(
                            ps[:, 0:512],
                            lhsT=xt_sb[d][:, T * 128:(T + 1) * 128],
                            rhs=wv_sb[d][:, 0:512],
                            start=(d == 0), stop=(d == DT - 1))
                        nc.tensor.matmul(
                            ps[:, 512:VW],
                            lhsT=xt_sb[d][:, T * 128:(T + 1) * 128],
                            rhs=wv_sb[d][:, 512:VW],
                            start=(d == 0), stop=(d == DT - 1))
                    vp = vp_p.tile([128, VW], bf, tag="vp")
                    nc.vector.tensor_copy(vp, ps[:, 0:VW])
                    v3 = vp.rearrange("p (h e) -> p h e", e=Dh + 1)
                    nc.vector.memset(v3[:, :, Dh:Dh + 1], 1.0)
                    vp_sb.append(vp)

                # ---- attention per pair ----------------------------------
                msa_sb = []
                for p in range(PAIRS):
                    msa = msa_p.tile([128, S], bf, tag="msa")
                    msa_sb.append(msa)
                    for c in range(SC):
                        po = [ps_av.tile([65, 512], f32, tag="ps_av",
                                         name=f"po{h}")
                              for h in range(2)]
                        for Tp in range(TT // 2):
                            pss = ps_sc.tile([128, 2048], f32, tag="ps_sc")
                            for j in range(2):
                                T = 2 * Tp + j
                                for h in range(2):
                                    nc.tensor.matmul(
                                        pss[:, (2 * j + h) * 512:
                                            (2 * j + h + 1) * 512],
                                        lhsT=kt_sb[p][h * 64:(h + 1) * 64,
                                                      T * 128:(T + 1) * 128],
                                        rhs=qt_sb[p][h * 64:(h + 1) * 64,
                                                     c * 512:(c + 1) * 512],
                                        start=True, stop=True,
                                        tile_position=(h * 64, 0))
                            at = a_p.tile([128, 2048], bf, tag="a")
                            nc.scalar.activation(at, pss, EXP, scale=0.125)
                            for j in range(2):
                                T = 2 * Tp + j
                                for h in range(2):
                                    nc.tensor.matmul(
                                        po[h],
                                        lhsT=vp_sb[T][
                                            :, (2 * p + h) * (Dh + 1):
                                            (2 * p + h + 1) * (Dh + 1)],
                                        rhs=at[:, (2 * j + h) * 512:
                                               (2 * j + h + 1) * 512],
                                        start=(T == 0), stop=(T == TT - 1))
                        for h in range(2):
                            r = r_p.tile([1, 512], f32, tag="r")
                            nc.vector.reciprocal(r, po[h][64:65, :])
                            rb = rb_p.tile([64, 512], f32, tag="rb")
                            nc.gpsimd.partition_broadcast(rb, r)
                            nc.vector.tensor_mul(
                                msa[h * 64:(h + 1) * 64,
                                    c * 512:(c + 1) * 512],
                                po[h][0:64, :], rb)

                # ---- output projection -----------------------------------
                for o in range(DT):
                    ps = ps_mm.tile([128, S], f32, tag="ps_mm")
                    for d in range(DT):
                        for c in range(SC):
                            nc.tensor.matmul(
                                ps[:, c * 512:(c + 1) * 512],
                                lhsT=wo_sb[d][:, o * 128:(o + 1) * 128],
                                rhs=msa_sb[d][:, c * 512:(c + 1) * 512],
                                start=(d == 0), stop=(d == DT - 1))
                    y = y_p.tile([128, S], f32, tag="y")
                    nc.vector.tensor_scalar_add(y, ps, bo_sb[:, o:o + 1])
                    nc.sync.dma_start(
                        out=YT[b, o * 128:(o + 1) * 128, :], in_=y)

    nc.compile()
    return nc


def _prep_inputs(X, Wq, bq, Wk, bk, Wv, bv, Wo, bo):
    bf16 = ml_dtypes.bfloat16
    X = np.asarray(X, dtype=np.float32)
    # per-core X^T: [core][BLOC, D, S]
    xt = np.ascontiguousarray(
        X.reshape(NCORE, BLOC, S, D).transpose(0, 1, 3, 2)).astype(bf16)
    wq = np.ascontiguousarray(
        np.asarray(Wq, np.float32).transpose(1, 0, 2).reshape(D, D)).astype(bf16)
    wk = np.ascontiguousarray(
        np.asarray(Wk, np.float32).transpose(1, 0, 2).reshape(D, D)).astype(bf16)
    wv = np.zeros((D, VW), np.float32)
    Wv = np.asarray(Wv, np.float32)
    for h in range(H):
        wv[:, h * (Dh + 1):h * (Dh + 1) + Dh] = Wv[h]
    wv = wv.astype(bf16)
    wo = np.asarray(Wo, np.float32).astype(bf16)
    bq2 = np.ascontiguousarray(
        np.asarray(bq, np.float32).reshape(PAIRS, 128).T)
    bo_eff = np.asarray(bo, np.float32) + \
        np.asarray(bv, np.float32).reshape(-1) @ np.asarray(Wo, np.float32)
    bo2 = np.ascontiguousarray(bo_eff.reshape(DT, 128).T.astype(np.float32))
    in_maps = [
        {"XT": xt[c], "WQ": wq, "WK": wk, "WV": wv, "WO": wo,
         "BQ": bq2, "BO": bo2}
        for c in range(NCORE)
    ]
    return in_maps


def _get_runner(repeats=1):
    """Build (once) a jitted SPMD runner over the 8 cores, modeled on
    bass2jax.run_bass_via_pjrt but cached so repeat calls don't re-trace."""
    key = ("runner", repeats)
    if key in _CACHE:
        return _CACHE[key]

    import jax
    import numpy as _np
    from jax.sharding import Mesh, PartitionSpec, NamedSharding
    from jax.experimental.shard_map import shard_map
    from concourse import mybir
    from concourse.bass2jax import (
        _bass_exec_p, install_neuronx_cc_hook, partition_id_tensor)

    nc = _build_program(repeats=repeats)
    install_neuronx_cc_hook()

    import concourse.mybir as _mybir
    in_names, out_names, out_avals, zero_shapes = [], [], [], []
    partition_name = (nc.partition_id_tensor.name
                      if nc.partition_id_tensor else None)
    for alloc in nc.m.functions[0].allocations:
        if not isinstance(alloc, _mybir.MemoryLocationSet):
            continue
        name = alloc.memorylocations[0].name
        if alloc.kind == "ExternalInput":
            if name != partition_name:
                in_names.append(name)
        elif alloc.kind == "ExternalOutput":
            shape = tuple(alloc.tensor_shape)
            dtype = _mybir.dt.np(alloc.dtype)
            out_names.append(name)
            out_avals.append(jax.core.ShapedArray(shape, dtype))
            zero_shapes.append((shape, dtype))
    n_params = len(in_names)
    n_outs = len(out_names)
    all_in_names = in_names + out_names
    if partition_name is not None:
        all_in_names = all_in_names + [partition_name]

    def _body(*args):
        operands = list(args)
        if partition_name is not None:
            operands.append(partition_id_tensor())
        outs = _bass_exec_p.bind(
            *operands,
            out_avals=tuple(out_avals),
            in_names=tuple(all_in_names),
            out_names=tuple(out_names),
            lowering_input_output_aliases=(),
            sim_require_finite=True,
            sim_require_nnan=True,
            nc=nc,
        )
        return tuple(outs)

    devices = jax.devices()[:NCORE]
    mesh = Mesh(_np.asarray(devices), ("core",))
    in_specs = (PartitionSpec("core"),) * (n_params + n_outs)
    out_specs = (PartitionSpec("core"),) * n_outs
    # NOTE: no donation — the kernel writes every output element, so the
    # custom call's self-allocated (uninit) output buffers are fine, and the
    # zero "output operand" arrays can be created once and reused across
    # calls instead of being shipped host->device (50 MB) per call.
    sharded = jax.jit(
        shard_map(_body, mesh=mesh, in_specs=in_specs, out_specs=out_specs,
                  check_rep=False),
        keep_unused=True)
    shard = NamedSharding(mesh, PartitionSpec("core"))
    import jax.numpy as jnp
    zeros_dev = [
        jax.device_put(_np.zeros((NCORE * s[0], *s[1:]), d), shard)
        for s, d in zero_shapes
    ]

    def put_inputs(in_maps):
        # concatenate along axis 0 (per-core stacking)
        concat = []
        for nm in in_names:
            arrs = [_np.asarray(in_maps[c][nm]) for c in range(NCORE)]
            concat.append(_np.concatenate(arrs, axis=0))
        return [jax.device_put(a, shard) for a in concat]

    def run(dev_inputs):
        outs = sharded(*dev_inputs, *zeros_dev)
        jax.block_until_ready(outs)
        return outs

    def unpack(outs):
        res = []
        for c in range(NCORE):
            d = {}
            for i, nm in enumerate(out_names):
                full = _np.asarray(outs[i])
                d[nm] = full.reshape(NCORE, *out_avals[i].shape)[c]
            res.append(d)
        return res

    _CACHE[key] = (put_inputs, run, unpack)
    return _CACHE[key]


def kernel(X, Wq, bq, Wk, bk, Wv, bv, Wo, bo):
    put_inputs, run, unpack = _get_runner()
    in_maps = _prep_inputs(X, Wq, bq, Wk, bk, Wv, bv, Wo, bo)
    dev_inputs = put_inputs(in_maps)
    outs = run(dev_inputs)
    res = unpack(outs)
    y = np.concatenate(
        [r["YT"].transpose(0, 2, 1) for r in res], axis=0)
    return np.ascontiguousarray(y.astype(np.float32))


# revision 17
# speedup vs baseline: 465.8915x; 2.0940x over previous
"""
Multi-head attention Trainium2 Bass kernel (B=16, S=1024, D=768, H=12, Dh=64).

Sharding: data parallel over batch — 8 cores x 2 batches each. Weights are
replicated; no collectives.

Per-core device algorithm (all matmuls bf16 with fp32 PSUM accumulation):
  1. QK^T projection: per head-pair tiles [Q^T_h0; Q^T_h1] and [K^T_h0; K^T_h1]
     of shape [128, S] (partition = head-dim e, stacked 2 heads), computed as
     lhsT = [W_h0 | W_h1] (stationary), rhs = X^T.  bq added on the PSUM->SBUF
     copy (per-partition scalar); bk is skipped entirely (constant-per-row
     terms cancel in softmax).
  2. V projection in [t, e] layout with a zero column per head that is later
     memset to 1 (V' = [V_h | 1]) -> AV matmul also produces softmax row-sums.
  3. scores^T tiles [t, s] via row-tiled (tile_position) pairs of K=64 matmuls
     (2 heads concurrently in the 128x128 array).  Softmax without max
     subtraction (scores/8 ~ N(0,1), exp is safe in fp32): ACT exp fused with
     the PSUM->SBUF copy, scale=1/8.
  4. AV: O'^T[e|rowsum, s] = V'^T A^T accumulated over t tiles.
  5. normalize: recip(rowsum) -> partition-broadcast -> multiply -> msa^T.
  6. out-projection Y^T = Wo^T msa^T + bo' where bo' = bo + bv_flat @ Wo
     (folded on host), written to DRAM as Y^T and transposed on host.
"""

import sys

for p in ("/opt/trn_rl_repo", "/root/.axon_site/_ro/trn_rl_repo"):
    if p not in sys.path:
        sys.path.insert(0, p)

import numpy as np
import ml_dtypes

B, S, D, H, Dh = 16, 1024, 768, 12, 64
NCORE = 8
BLOC = B // NCORE          # 2 batches per core
PAIRS = H // 2             # 6 head pairs
DT = D // 128              # 6 d-tiles (contraction tiles)
TT = S // 128              # 8 t-tiles
SC = S // 512              # 2 s-chunks
VW = H * (Dh + 1)          # 780: V' width incl. ones columns

_CACHE = {}


def _build_program(repeats=1):
    import concourse.tile as tile
    from concourse import bacc, mybir

    bf = mybir.dt.bfloat16
    f32 = mybir.dt.float32
    EXP = mybir.ActivationFunctionType.Exp

    nc = bacc.Bacc("TRN2", target_bir_lowering=False, debug=False,
                   num_devices=NCORE)

    XT = nc.dram_tensor("XT", [BLOC, D, S], bf, kind="ExternalInput").ap()
    WQ = nc.dram_tensor("WQ", [D, D], bf, kind="ExternalInput").ap()
    WK = nc.dram_tensor("WK", [D, D], bf, kind="ExternalInput").ap()
    WV = nc.dram_tensor("WV", [D, VW], bf, kind="ExternalInput").ap()
    WO = nc.dram_tensor("WO", [D, D], bf, kind="ExternalInput").ap()
    BQ = nc.dram_tensor("BQ", [128, PAIRS], f32, kind="ExternalInput").ap()
    BO = nc.dram_tensor("BO", [128, DT], f32, kind="ExternalInput").ap()
    YT = nc.dram_tensor("YT", [BLOC, D, S], f32, kind="ExternalOutput").ap()

    with tile.TileContext(nc) as tc:
        import contextlib
        with contextlib.ExitStack() as ctx:
            consts = ctx.enter_context(tc.tile_pool(name="consts", bufs=1))
            xt_p = ctx.enter_context(tc.tile_pool(name="xt", bufs=2 * DT))
            qk_p = ctx.enter_context(tc.tile_pool(name="qk", bufs=3 * PAIRS))
            vp_p = ctx.enter_context(tc.tile_pool(name="vp", bufs=2 * TT))
            a_p = ctx.enter_context(tc.tile_pool(name="a", bufs=3))
            msa_p = ctx.enter_context(tc.tile_pool(name="msa", bufs=2 * DT))
            y_p = ctx.enter_context(tc.tile_pool(name="y", bufs=3))
            r_p = ctx.enter_context(tc.tile_pool(name="r", bufs=4))
            rb_p = ctx.enter_context(tc.tile_pool(name="rb", bufs=4))
            ps_mm = ctx.enter_context(
                tc.tile_pool(name="ps_mm", bufs=2, space="PSUM"))
            ps_av = ctx.enter_context(
                tc.tile_pool(name="ps_av", bufs=4, space="PSUM"))

            # ---- resident weights / biases -------------------------------
            wq_sb = []
            wk_sb = []
            wv_sb = []
            wo_sb = []
            for d in range(DT):
                t = consts.tile([128, D], bf, tag=f"wq{d}")
                nc.sync.dma_start(out=t, in_=WQ[d * 128:(d + 1) * 128, :])
                wq_sb.append(t)
                t = consts.tile([128, D], bf, tag=f"wk{d}")
                nc.sync.dma_start(out=t, in_=WK[d * 128:(d + 1) * 128, :])
                wk_sb.append(t)
                t = consts.tile([128, VW], bf, tag=f"wv{d}")
                nc.sync.dma_start(out=t, in_=WV[d * 128:(d + 1) * 128, :])
                wv_sb.append(t)
                t = consts.tile([128, D], bf, tag=f"wo{d}")
                nc.sync.dma_start(out=t, in_=WO[d * 128:(d + 1) * 128, :])
                wo_sb.append(t)
            bq_sb = consts.tile([128, PAIRS], f32, tag="bq")
            nc.sync.dma_start(out=bq_sb, in_=BQ)
            bo_sb = consts.tile([128, DT], f32, tag="bo")
            nc.sync.dma_start(out=bo_sb, in_=BO)

            for b in [b for _ in range(repeats) for b in range(BLOC)]:
                # ---- load X^T tiles --------------------------------------
                xt_sb = []
                for d in range(DT):
                    t = xt_p.tile([128, S], bf, tag="xt")
                    nc.sync.dma_start(
                        out=t, in_=XT[b, d * 128:(d + 1) * 128, :])
                    xt_sb.append(t)

                # ---- QK^T projections per pair ---------------------------
                qt_sb = []
                kt_sb = []
                for p in range(PAIRS):
                    for kind in ("q", "k"):
                        w = wq_sb if kind == "q" else wk_sb
                        ps = ps_mm.tile([128, S], f32, tag="ps_mm")
                        for d in range(DT):
                            for c in range(SC):
                                nc.tensor.matmul(
                                    ps[:, c * 512:(c + 1) * 512],
                                    lhsT=w[d][:, p * 128:(p + 1) * 128],
                                    rhs=xt_sb[d][:, c * 512:(c + 1) * 512],
                                    start=(d == 0), stop=(d == DT - 1))
                        out = qk_p.tile([128, S], bf, tag="qk")
                        if kind == "q":
                            nc.vector.tensor_scalar_add(
                                out, ps, bq_sb[:, p:p + 1])
                            qt_sb.append(out)
                        else:
                            nc.vector.tensor_copy(out, ps)
                            kt_sb.append(out)

                # ---- V' in [t, e] layout (with zero cols for ones) -------
                vp_sb = []
                for T in range(TT):
                    ps = ps_mm.tile([128, S], f32, tag="ps_mm")
                    for d in range(DT):
                        nc.tensor.matmul(
                            ps[:, 0:512],
                            lhsT=xt_sb[d][:, T * 128:(T + 1) * 128],
                            rhs=wv_sb[d][:, 0:512],
                            start=(d == 0), stop=(d == DT - 1))
                        nc.tensor.matmul(
                            ps[:, 512:VW],
                            lhsT=xt_sb[d][:, T * 128:(T + 1) * 128],
                            rhs=wv_sb[d][:, 512:VW],
                            start=(d == 0), stop=(d == DT - 1))
                    vp = vp_p.tile([128, VW], bf, tag="vp")
                    nc.vector.tensor_copy(vp, ps[:, 0:VW])
                    v3 = vp.rearrange("p (h e) -> p h e", e=Dh + 1)
                    nc.vector.memset(v3[:, :, Dh:Dh + 1], 1.0)
                    vp_sb.append(vp)

                # ---- attention per pair ----------------------------------
                msa_sb = []
                for p in range(PAIRS):
                    msa = msa_p.tile([128, S], bf, tag="msa")
                    msa_sb.append(msa)
                    for c in range(SC):
                        po = [ps_av.tile([65, 512], f32, tag="ps_av",
                                         name=f"po{h}")
                              for h in range(2)]
                        for T in range(TT):
                            pss = ps_mm.tile([128, 1024], f32, tag="ps_mm", name="pss")
                            for h in range(2):
                                nc.tensor.matmul(
                                    pss[:, h * 512:(h + 1) * 512],
                                    lhsT=kt_sb[p][h * 64:(h + 1) * 64,
                                                  T * 128:(T + 1) * 128],
                                    rhs=qt_sb[p][h * 64:(h + 1) * 64,
                                                 c * 512:(c + 1) * 512],
                                    start=True, stop=True,
                                    tile_position=(h * 64, 0))
                            at = a_p.tile([128, 1024], bf, tag="a")
                            nc.scalar.activation(at, pss, EXP, scale=0.125)
                            for h in range(2):
                                nc.tensor.matmul(
                                    po[h],
                                    lhsT=vp_sb[T][
                                        :, (2 * p + h) * (Dh + 1):
                                        (2 * p + h + 1) * (Dh + 1)],
                                    rhs=at[:, h * 512:(h + 1) * 512],
                                    start=(T == 0), stop=(T == TT - 1))
                        for h in range(2):
                            r = r_p.tile([1, 512], f32, tag="r")
                            nc.vector.reciprocal(r, po[h][64:65, :])
                            rb = rb_p.tile([64, 512], f32, tag="rb")
                            nc.gpsimd.partition_broadcast(rb, r)
                            nc.vector.tensor_mul(
                                msa[h * 64:(h + 1) * 64,
                                    c * 512:(c + 1) * 512],
                                po[h][0:64, :], rb)

                # ---- output projection -----------------------------------
                for o in range(DT):
                    ps = ps_mm.tile([128, S], f32, tag="ps_mm")
                    for d in range(DT):
                        for c in range(SC):
                            nc.tensor.matmul(
                                ps[:, c * 512:(c + 1) * 512],
                                lhsT=wo_sb[d][:, o * 128:(o + 1) * 128],
                                rhs=msa_sb[d][:, c * 512:(c + 1) * 512],
                                start=(d == 0), stop=(d == DT - 1))
                    y = y_p.tile([128, S], f32, tag="y")
                    nc.vector.tensor_scalar_add(y, ps, bo_sb[:, o:o + 1])
                    nc.sync.dma_start(
                        out=YT[b, o * 128:(o + 1) * 128, :], in_=y)

    nc.compile()
    return nc


def _prep_inputs(X, Wq, bq, Wk, bk, Wv, bv, Wo, bo):
    bf16 = ml_dtypes.bfloat16
    X = np.asarray(X, dtype=np.float32)
    # per-core X^T: [core][BLOC, D, S]
    xt = np.ascontiguousarray(
        X.reshape(NCORE, BLOC, S, D).transpose(0, 1, 3, 2)).astype(bf16)
    wq = np.ascontiguousarray(
        np.asarray(Wq, np.float32).transpose(1, 0, 2).reshape(D, D)).astype(bf16)
    wk = np.ascontiguousarray(
        np.asarray(Wk, np.float32).transpose(1, 0, 2).reshape(D, D)).astype(bf16)
    wv = np.zeros((D, VW), np.float32)
    Wv = np.asarray(Wv, np.float32)
    for h in range(H):
        wv[:, h * (Dh + 1):h * (Dh + 1) + Dh] = Wv[h]
    wv = wv.astype(bf16)
    wo = np.asarray(Wo, np.float32).astype(bf16)
    bq2 = np.ascontiguousarray(
        np.asarray(bq, np.float32).reshape(PAIRS, 128).T)
    bo_eff = np.asarray(bo, np.float32) + \
        np.asarray(bv, np.float32).reshape(-1) @ np.asarray(Wo, np.float32)
    bo2 = np.ascontiguousarray(bo_eff.reshape(DT, 128).T.astype(np.float32))
    in_maps = [
        {"XT": xt[c], "WQ": wq, "WK": wk, "WV": wv, "WO": wo,
         "BQ": bq2, "BO": bo2}
        for c in range(NCORE)
    ]
    return in_maps


def _get_runner(repeats=1):
    """Build (once) a jitted SPMD runner over the 8 cores, modeled on
    bass2jax.run_bass_via_pjrt but cached so repeat calls don't re-trace."""
    key = ("runner", repeats)
    if key in _CACHE:
        return _CACHE[key]

    import jax
    import numpy as _np
    from jax.sharding import Mesh, PartitionSpec, NamedSharding
    from jax.experimental.shard_map import shard_map
    from concourse import mybir
    from concourse.bass2jax import (
        _bass_exec_p, install_neuronx_cc_hook, partition_id_tensor)

    nc = _build_program(repeats=repeats)
    install_neuronx_cc_hook()

    import concourse.mybir as _mybir
    in_names, out_names, out_avals, zero_shapes = [], [], [], []
    partition_name = (nc.partition_id_tensor.name
                      if nc.partition_id_tensor else None)
    for alloc in nc.m.functions[0].allocations:
        if not isinstance(alloc, _mybir.MemoryLocationSet):
            continue
        name = alloc.memorylocations[0].name
        if alloc.kind == "ExternalInput":
            if name != partition_name:
                in_names.append(name)
        elif alloc.kind == "ExternalOutput":
            shape = tuple(alloc.tensor_shape)
            dtype = _mybir.dt.np(alloc.dtype)
            out_names.append(name)
            out_avals.append(jax.core.ShapedArray(shape, dtype))
            zero_shapes.append((shape, dtype))
    n_params = len(in_names)
    n_outs = len(out_names)
    all_in_names = in_names + out_names
    if partition_name is not None:
        all_in_names = all_in_names + [partition_name]

    def _body(*args):
        operands = list(args)
        if partition_name is not None:
            operands.append(partition_id_tensor())
        outs = _bass_exec_p.bind(
            *operands,
            out_avals=tuple(out_avals),
            in_names=tuple(all_in_names),
            out_names=tuple(out_names),
            lowering_input_output_aliases=(),
            sim_require_finite=True,
            sim_require_nnan=True,
            nc=nc,
        )
        return tuple(outs)

    devices = jax.devices()[:NCORE]
    mesh = Mesh(_np.asarray(devices), ("core",))
    in_specs = (PartitionSpec("core"),) * (n_params + n_outs)
    out_specs = (PartitionSpec("core"),) * n_outs
    # NOTE: no donation — the kernel writes every output element, so the
    # custom call's self-allocated (uninit) output buffers are fine, and the
    # zero "output operand" arrays can be created once and reused across
    # calls instead of being shipped host->device (50 MB) per call.
    sharded = jax.jit(
        shard_map(_body, mesh=mesh, in_specs=in_specs, out_specs=out_specs,
                  check_rep=False),
        keep_unused=True)
    shard = NamedSharding(mesh, PartitionSpec("core"))
    import jax.numpy as jnp
    zeros_dev = [
        jax.device_put(_np.zeros((NCORE * s[0], *s[1:]), d), shard)
        for s, d in zero_shapes
    ]

    def put_inputs(in_maps):
        # concatenate along axis 0 (per-core stacking)
        concat = []
        for nm in in_names:
            arrs = [_np.asarray(in_maps[c][nm]) for c in range(NCORE)]
            concat.append(_np.concatenate(arrs, axis=0))
        return [jax.device_put(a, shard) for a in concat]

    def run(dev_inputs):
        outs = sharded(*dev_inputs, *zeros_dev)
        jax.block_until_ready(outs)
        return outs

    def unpack(outs):
        res = []
        for c in range(NCORE):
            d = {}
            for i, nm in enumerate(out_names):
                full = _np.asarray(outs[i])
                d[nm] = full.reshape(NCORE, *out_avals[i].shape)[c]
            res.append(d)
        return res

    _CACHE[key] = (put_inputs, run, unpack)
    return _CACHE[key]


def kernel(X, Wq, bq, Wk, bk, Wv, bv, Wo, bo):
    put_inputs, run, unpack = _get_runner()
    in_maps = _prep_inputs(X, Wq, bq, Wk, bk, Wv, bv, Wo, bo)
    dev_inputs = put_inputs(in_maps)
    outs = run(dev_inputs)
    res = unpack(outs)
    y = np.concatenate(
        [r["YT"].transpose(0, 2, 1) for r in res], axis=0)
    return np.ascontiguousarray(y.astype(np.float32))


# revision 18
# speedup vs baseline: 1341.9954x; 2.8805x over previous
"""
Multi-head attention Trainium2 Bass kernel (B=16, S=1024, D=768, H=12, Dh=64).

Sharding: data parallel over batch — 8 cores x 2 batches each. Weights are
replicated; no collectives.

Per-core device algorithm (all matmuls bf16 with fp32 PSUM accumulation):
  1. QK^T projection: per head-pair tiles [Q^T_h0; Q^T_h1] and [K^T_h0; K^T_h1]
     of shape [128, S] (partition = head-dim e, stacked 2 heads), computed as
     lhsT = [W_h0 | W_h1] (stationary), rhs = X^T.  bq added on the PSUM->SBUF
     copy (per-partition scalar); bk is skipped entirely (constant-per-row
     terms cancel in softmax).
  2. V projection in [t, e] layout with a zero column per head that is later
     memset to 1 (V' = [V_h | 1]) -> AV matmul also produces softmax row-sums.
  3. scores^T tiles [t, s] via row-tiled (tile_position) pairs of K=64 matmuls
     (2 heads concurrently in the 128x128 array).  Softmax without max
     subtraction (scores/8 ~ N(0,1), exp is safe in fp32): ACT exp fused with
     the PSUM->SBUF copy, scale=1/8.
  4. AV: O'^T[e|rowsum, s] = V'^T A^T accumulated over t tiles.
  5. normalize: recip(rowsum) -> partition-broadcast -> multiply -> msa^T.
  6. out-projection Y^T = Wo^T msa^T + bo' where bo' = bo + bv_flat @ Wo
     (folded on host), written to DRAM as Y^T and transposed on host.
"""

import sys

for p in ("/opt/trn_rl_repo", "/root/.axon_site/_ro/trn_rl_repo"):
    if p not in sys.path:
        sys.path.insert(0, p)

import numpy as np
import ml_dtypes

B, S, D, H, Dh = 16, 1024, 768, 12, 64
NCORE = 8
BLOC = B // NCORE          # 2 batches per core
PAIRS = H // 2             # 6 head pairs
DT = D // 128              # 6 d-tiles (contraction tiles)
TT = S // 128              # 8 t-tiles
SC = S // 512              # 2 s-chunks
VW = H * (Dh + 1)          # 780: V' width incl. ones columns

_CACHE = {}


def _build_program(repeats=1):
    import concourse.tile as tile
    from concourse import bacc, mybir

    bf = mybir.dt.bfloat16
    f32 = mybir.dt.float32
    EXP = mybir.ActivationFunctionType.Exp

    nc = bacc.Bacc("TRN2", target_bir_lowering=False, debug=False,
                   num_devices=NCORE)

    XT = nc.dram_tensor("XT", [BLOC, D, S], bf, kind="ExternalInput").ap()
    WQ = nc.dram_tensor("WQ", [D, D], bf, kind="ExternalInput").ap()
    WK = nc.dram_tensor("WK", [D, D], bf, kind="ExternalInput").ap()
    WV = nc.dram_tensor("WV", [D, VW], bf, kind="ExternalInput").ap()
    WO = nc.dram_tensor("WO", [D, D], bf, kind="ExternalInput").ap()
    BQ = nc.dram_tensor("BQ", [128, PAIRS], f32, kind="ExternalInput").ap()
    BO = nc.dram_tensor("BO", [128, DT], f32, kind="ExternalInput").ap()
    YT = nc.dram_tensor("YT", [BLOC, D, S], f32, kind="ExternalOutput").ap()

    with tile.TileContext(nc) as tc:
        import contextlib
        with contextlib.ExitStack() as ctx:
            consts = ctx.enter_context(tc.tile_pool(name="consts", bufs=1))
            xt_p = ctx.enter_context(tc.tile_pool(name="xt", bufs=2 * DT))
            qk_p = ctx.enter_context(tc.tile_pool(name="qk", bufs=3 * PAIRS))
            vp_p = ctx.enter_context(tc.tile_pool(name="vp", bufs=2 * TT))
            a_p = ctx.enter_context(tc.tile_pool(name="a", bufs=3))
            msa_p = ctx.enter_context(tc.tile_pool(name="msa", bufs=2 * DT))
            y_p = ctx.enter_context(tc.tile_pool(name="y", bufs=3))
            r_p = ctx.enter_context(tc.tile_pool(name="r", bufs=4))
            rb_p = ctx.enter_context(tc.tile_pool(name="rb", bufs=4))
            ps_mm = ctx.enter_context(
                tc.tile_pool(name="ps_mm", bufs=2, space="PSUM"))
            ps_av = ctx.enter_context(
                tc.tile_pool(name="ps_av", bufs=4, space="PSUM"))

            # ---- resident weights / biases -------------------------------
            wq_sb = []
            wk_sb = []
            wv_sb = []
            wo_sb = []
            for d in range(DT):
                t = consts.tile([128, D], bf, tag=f"wq{d}")
                nc.sync.dma_start(out=t, in_=WQ[d * 128:(d + 1) * 128, :])
                wq_sb.append(t)
                t = consts.tile([128, D], bf, tag=f"wk{d}")
                nc.sync.dma_start(out=t, in_=WK[d * 128:(d + 1) * 128, :])
                wk_sb.append(t)
                t = consts.tile([128, VW], bf, tag=f"wv{d}")
                nc.sync.dma_start(out=t, in_=WV[d * 128:(d + 1) * 128, :])
                wv_sb.append(t)
                t = consts.tile([128, D], bf, tag=f"wo{d}")
                nc.sync.dma_start(out=t, in_=WO[d * 128:(d + 1) * 128, :])
                wo_sb.append(t)
            bq_sb = consts.tile([128, PAIRS], f32, tag="bq")
            nc.sync.dma_start(out=bq_sb, in_=BQ)
            bo_sb = consts.tile([128, DT], f32, tag="bo")
            nc.sync.dma_start(out=bo_sb, in_=BO)

            for b in [b for _ in range(repeats) for b in range(BLOC)]:
                # ---- load X^T tiles --------------------------------------
                xt_sb = []
                for d in range(DT):
                    t = xt_p.tile([128, S], bf, tag="xt")
                    nc.sync.dma_start(
                        out=t, in_=XT[b, d * 128:(d + 1) * 128, :])
                    xt_sb.append(t)

                # ---- QK^T projections per pair ---------------------------
                qt_sb = []
                kt_sb = []
                for p in range(PAIRS):
                    for kind in ("q", "k"):
                        w = wq_sb if kind == "q" else wk_sb
                        ps = ps_mm.tile([128, S], f32, tag="ps_mm")
                        for d in range(DT):
                            for c in range(SC):
                                nc.tensor.matmul(
                                    ps[:, c * 512:(c + 1) * 512],
                                    lhsT=w[d][:, p * 128:(p + 1) * 128],
                                    rhs=xt_sb[d][:, c * 512:(c + 1) * 512],
                                    start=(d == 0), stop=(d == DT - 1))
                        out = qk_p.tile([128, S], bf, tag="qk")
                        if kind == "q":
                            nc.vector.tensor_scalar_add(
                                out, ps, bq_sb[:, p:p + 1])
                            qt_sb.append(out)
                        else:
                            nc.vector.tensor_copy(out, ps)
                            kt_sb.append(out)

                # ---- V' in [t, e] layout (with zero cols for ones) -------
                vp_sb = []
                for T in range(TT):
                    ps = ps_mm.tile([128, S], f32, tag="ps_mm")
                    for d in range(DT):
                        nc.tensor.matmul(
                            ps[:, 0:512],
                            lhsT=xt_sb[d][:, T * 128:(T + 1) * 128],
                            rhs=wv_sb[d][:, 0:512],
                            start=(d == 0), stop=(d == DT - 1))
                        nc.tensor.matmul(
                            ps[:, 512:VW],
                            lhsT=xt_sb[d][:, T * 128:(T + 1) * 128],
                            rhs=wv_sb[d][:, 512:VW],
                            start=(d == 0), stop=(d == DT - 1))
                    vp = vp_p.tile([128, VW], bf, tag="vp")
                    nc.vector.tensor_copy(vp, ps[:, 0:VW])
                    v3 = vp.rearrange("p (h e) -> p h e", e=Dh + 1)
                    nc.vector.memset(v3[:, :, Dh:Dh + 1], 1.0)
                    vp_sb.append(vp)

                # ---- attention per pair ----------------------------------
                msa_sb = []
                for p in range(PAIRS):
                    msa = msa_p.tile([128, S], bf, tag="msa")
                    msa_sb.append(msa)
                    for c in range(SC):
                        po = [ps_av.tile([65, 512], f32, tag="ps_av",
                                         name=f"po{h}")
                              for h in range(2)]
                        for T in range(TT):
                            pss = ps_mm.tile([128, 1024], f32, tag="ps_mm", name="pss")
                            for h in range(2):
                                nc.tensor.matmul(
                                    pss[:, h * 512:(h + 1) * 512],
                                    lhsT=kt_sb[p][h * 64:(h + 1) * 64,
                                                  T * 128:(T + 1) * 128],
                                    rhs=qt_sb[p][h * 64:(h + 1) * 64,
                                                 c * 512:(c + 1) * 512],
                                    start=True, stop=True,
                                    tile_position=(h * 64, 0))
                            at = a_p.tile([128, 1024], bf, tag="a")
                            nc.scalar.activation(at, pss, EXP, scale=0.125)
                            for h in range(2):
                                nc.tensor.matmul(
                                    po[h],
                                    lhsT=vp_sb[T][
                                        :, (2 * p + h) * (Dh + 1):
                                        (2 * p + h + 1) * (Dh + 1)],
                                    rhs=at[:, h * 512:(h + 1) * 512],
                                    start=(T == 0), stop=(T == TT - 1))
                        for h in range(2):
                            r = r_p.tile([1, 512], f32, tag="r")
                            nc.vector.reciprocal(r, po[h][64:65, :])
                            rb = rb_p.tile([64, 512], f32, tag="rb")
                            nc.gpsimd.partition_broadcast(rb, r)
                            nc.vector.tensor_mul(
                                msa[h * 64:(h + 1) * 64,
                                    c * 512:(c + 1) * 512],
                                po[h][0:64, :], rb)

                # ---- output projection -----------------------------------
                for o in range(DT):
                    ps = ps_mm.tile([128, S], f32, tag="ps_mm")
                    for d in range(DT):
                        for c in range(SC):
                            nc.tensor.matmul(
                                ps[:, c * 512:(c + 1) * 512],
                                lhsT=wo_sb[d][:, o * 128:(o + 1) * 128],
                                rhs=msa_sb[d][:, c * 512:(c + 1) * 512],
                                start=(d == 0), stop=(d == DT - 1))
                    y = y_p.tile([128, S], f32, tag="y")
                    nc.vector.tensor_scalar_add(y, ps, bo_sb[:, o:o + 1])
                    nc.sync.dma_start(
                        out=YT[b, o * 128:(o + 1) * 128, :], in_=y)

    nc.compile()
    return nc


def _prep_inputs(X, Wq, bq, Wk, bk, Wv, bv, Wo, bo):
    bf16 = ml_dtypes.bfloat16
    X = np.asarray(X, dtype=np.float32)
    # per-core X^T: [core][BLOC, D, S]
    xt = np.ascontiguousarray(
        X.reshape(NCORE, BLOC, S, D).transpose(0, 1, 3, 2)).astype(bf16)
    wq = np.ascontiguousarray(
        np.asarray(Wq, np.float32).transpose(1, 0, 2).reshape(D, D)).astype(bf16)
    wk = np.ascontiguousarray(
        np.asarray(Wk, np.float32).transpose(1, 0, 2).reshape(D, D)).astype(bf16)
    wv = np.zeros((D, VW), np.float32)
    Wv = np.asarray(Wv, np.float32)
    for h in range(H):
        wv[:, h * (Dh + 1):h * (Dh + 1) + Dh] = Wv[h]
    wv = wv.astype(bf16)
    wo = np.asarray(Wo, np.float32).astype(bf16)
    bq2 = np.ascontiguousarray(
        np.asarray(bq, np.float32).reshape(PAIRS, 128).T)
    bo_eff = np.asarray(bo, np.float32) + \
        np.asarray(bv, np.float32).reshape(-1) @ np.asarray(Wo, np.float32)
    bo2 = np.ascontiguousarray(bo_eff.reshape(DT, 128).T.astype(np.float32))
    in_maps = [
        {"XT": xt[c], "WQ": wq, "WK": wk, "WV": wv, "WO": wo,
         "BQ": bq2, "BO": bo2}
        for c in range(NCORE)
    ]
    return in_maps


def _get_runner(repeats=1):
    """Build (once) a jitted SPMD runner over the 8 cores, modeled on
    bass2jax.run_bass_via_pjrt but cached so repeat calls don't re-trace."""
    key = ("runner", repeats)
    if key in _CACHE:
        return _CACHE[key]

    import jax
    import numpy as _np
    from jax.sharding import Mesh, PartitionSpec, NamedSharding
    from jax.experimental.shard_map import shard_map
    from concourse import mybir
    from concourse.bass2jax import (
        _bass_exec_p, install_neuronx_cc_hook, partition_id_tensor)

    nc = _build_program(repeats=repeats)
    install_neuronx_cc_hook()

    import concourse.mybir as _mybir
    in_names, out_names, out_avals, zero_shapes = [], [], [], []
    partition_name = (nc.partition_id_tensor.name
                      if nc.partition_id_tensor else None)
    for alloc in nc.m.functions[0].allocations:
        if not isinstance(alloc, _mybir.MemoryLocationSet):
            continue
        name = alloc.memorylocations[0].name
        if alloc.kind == "ExternalInput":
            if name != partition_name:
                in_names.append(name)
        elif alloc.kind == "ExternalOutput":
            shape = tuple(alloc.tensor_shape)
            dtype = _mybir.dt.np(alloc.dtype)
            out_names.append(name)
            out_avals.append(jax.core.ShapedArray(shape, dtype))
            zero_shapes.append((shape, dtype))
    n_params = len(in_names)
    n_outs = len(out_names)
    all_in_names = in_names + out_names
    if partition_name is not None:
        all_in_names = all_in_names + [partition_name]

    def _body(*args):
        operands = list(args)
        if partition_name is not None:
            operands.append(partition_id_tensor())
        outs = _bass_exec_p.bind(
            *operands,
            out_avals=tuple(out_avals),
            in_names=tuple(all_in_names),
            out_names=tuple(out_names),
            lowering_input_output_aliases=(),
            sim_require_finite=True,
            sim_require_nnan=True,
            nc=nc,
        )
        return tuple(outs)

    devices = jax.devices()[:NCORE]
    mesh = Mesh(_np.asarray(devices), ("core",))
    in_specs = (PartitionSpec("core"),) * (n_params + n_outs)
    out_specs = (PartitionSpec("core"),) * n_outs
    # NOTE: no donation — the kernel writes every output element, so the
    # custom call's self-allocated (uninit) output buffers are fine, and the
    # zero "output operand" arrays can be created once and reused across
    # calls instead of being shipped host->device (50 MB) per call.
    sharded = jax.jit(
        shard_map(_body, mesh=mesh, in_specs=in_specs, out_specs=out_specs,
                  check_rep=False),
        keep_unused=True)
    shard = NamedSharding(mesh, PartitionSpec("core"))
    import jax.numpy as jnp
    zeros_dev = [
        jax.device_put(_np.zeros((NCORE * s[0], *s[1:]), d), shard)
        for s, d in zero_shapes
    ]

    def put_inputs(in_maps):
        # concatenate along axis 0 (per-core stacking)
        concat = []
        for nm in in_names:
            arrs = [_np.asarray(in_maps[c][nm]) for c in range(NCORE)]
            concat.append(_np.concatenate(arrs, axis=0))
        return [jax.device_put(a, shard) for a in concat]

    _CACHE[("sharded", repeats)] = (sharded, zeros_dev)

    def run(dev_inputs):
        outs = sharded(*dev_inputs, *zeros_dev)
        jax.block_until_ready(outs)
        return outs

    def unpack(outs):
        res = []
        for c in range(NCORE):
            d = {}
            for i, nm in enumerate(out_names):
                full = _np.asarray(outs[i])
                d[nm] = full.reshape(NCORE, *out_avals[i].shape)[c]
            res.append(d)
        return res

    _CACHE[key] = (put_inputs, run, unpack)
    return _CACHE[key]


def kernel(X, Wq, bq, Wk, bk, Wv, bv, Wo, bo):
    put_inputs, run, unpack = _get_runner()
    in_maps = _prep_inputs(X, Wq, bq, Wk, bk, Wv, bv, Wo, bo)
    dev_inputs = put_inputs(in_maps)
    outs = run(dev_inputs)
    res = unpack(outs)
    y = np.concatenate(
        [r["YT"].transpose(0, 2, 1) for r in res], axis=0)
    return np.ascontiguousarray(y.astype(np.float32))
